# revision 29
# baseline (speedup 1.0000x reference)
"""Trainium2 Bass kernel for nn_Conv_block_57690000720236.

Reference computation (per batch image b):
  - 3x3 SAME conv "high" branch: 64ch -> 64ch
  - low branch: 3x3 conv 64ch -> 16ch, then 1x1 conv 16ch -> 64ch
  - output position (b,y,x) takes the high value if its flat index is in
    mask_idx, the low value if in inv_mask_idx (inv wins on overlap), 0 if
    in neither.

Strategy (8 NeuronCores, data-parallel over batch):
  - Core b computes BOTH branches densely for image b; the low branch is
    folded on the host (W_low = w2 @ w1) so both branches are 3x3 convs,
    evaluated together as M=128 output columns (64 high + 64 low).
  - The padded image lives in SBUF as FLAT [row*130+col] buffers, so every
    conv tap is a column-offset view (no per-tile shifted copies):
      bufA: partitions 0-63 = I, 64-127 = I shifted one row (+130)
      bufB: partitions 0-63 = I+129,  64-127 = I+130
    5 matmul passes per 4-row chunk (K=576 padded to 5x128):
      pass 1-3: taps (ky0,c)+(ky1,c) from bufA view rows l0..l0+3, cols c
      pass 4:   tap (ky2,kx2) K=64 (zero-padded weights) from bufA rows
                l0+2..l0+5 cols 2
      pass 5:   taps (ky2,kx0)+(ky2,kx1) from bufB rows l0+1..l0+4 cols 1
  - Input DMA is 3 image copies (bufA both halves + bufB top from HBM);
    bufB's bottom half equals bufA's bottom half verbatim and is copied
    on-chip by ACT/DVE (same partitions), saving 2.2MB of HBM traffic.
  - Routing per 1024-col granule: ACT evicts PSUM to SBUF bf16, a gpsimd
    (SWDGE) DMA moves the low half across partitions into the output
    buffer, one DVE copy_predicated overlays the high half per mask.
    Moves ride the otherwise-idle gpsimd queue so they never sit behind
    the input loads (sync) or the output stores (scalar).
"""

import numpy as np
import ml_dtypes

import concourse.bacc as bacc
import concourse.mybir as mybir
import concourse.tile as tile
from concourse.bass_utils import run_bass_kernel_spmd

B, CIN, H, W = 8, 64, 128, 128
COUT, KER = 64, 3
NPOS = H * W                 # 16384 positions per core
WP = W + 2                   # padded row length 130
HP = H + 2                   # padded rows 130
L = HP * WP                  # flat padded image length 16900
N_TILES = 8                  # image row-tiles
TROWS = H // N_TILES         # 16 output rows per tile
CHUNK_ROWS = 4               # output rows per matmul chunk
CHUNK = CHUNK_ROWS * W       # 512 positions per chunk
GRAN = 2 * CHUNK             # 1024 positions per merge granule
TILE_POS = TROWS * W         # 2048 positions per tile
BLK = TROWS * WP             # input block cols per tile (16 rows x 130)
F32 = mybir.dt.float32
BF16 = mybir.dt.bfloat16
U8 = mybir.dt.uint8
U16 = mybir.dt.uint16
OUTDT = mybir.dt.bfloat16    # on-chip merge + writeback dtype
WBLK = 5 * 128               # weight blob: 5 matmul blocks


def _build_program(need_zero_fix: bool):
    nc = bacc.Bacc("TRN2", target_bir_lowering=False, debug=False, num_devices=B)

    a_d = nc.dram_tensor("hbma", [128, L], BF16, kind="ExternalInput")
    b_d = nc.dram_tensor("hbmb", [COUT, L], BF16, kind="ExternalInput")
    w_d = nc.dram_tensor("wblob", [128, WBLK], BF16, kind="ExternalInput")
    m_d = nc.dram_tensor("mhigh", [COUT, NPOS], U8, kind="ExternalInput")
    if need_zero_fix:
        mz_d = nc.dram_tensor("mzero", [COUT, NPOS], U8, kind="ExternalInput")
    out_d = nc.dram_tensor("out", [COUT, NPOS], OUTDT, kind="ExternalOutput")

    # A-blocks: 16 image rows each, last block takes the 2 padded tail rows.
    ablk = [(j * BLK, min((j + 1) * BLK, L) if j < N_TILES - 1 else L)
            for j in range(N_TILES)]
    # B cols are only ever read for x in [131, 129*130); shift blocks by one
    # row so tile t's pass-5 window sits inside block t alone.
    bblk = [(j * BLK + WP, min((j + 1) * BLK + WP, 129 * WP))
            for j in range(N_TILES)]

    with tile.TileContext(nc) as tc:
        with (
            tc.tile_pool(name="const", bufs=1) as cpool,
            tc.tile_pool(name="outp", bufs=4) as opool,
            tc.tile_pool(name="evp", bufs=4) as epool,
            tc.tile_pool(name="psum", bufs=3, space="PSUM") as pspool,
            tc.tile_pool(name="psumw", bufs=1, space="PSUM") as pwpool,
        ):
            wt = cpool.tile([128, WBLK], BF16, tag="wblob")
            nc.sync.dma_start(wt[:], w_d[:])

            at = cpool.tile([128, L], BF16, tag="bufa")
            bt = cpool.tile([128, L], BF16, tag="bufb")

            def load_a(j):
                c0, c1 = ablk[j]
                nc.sync.dma_start(at[:, c0:c1], a_d[:, c0:c1])

            def load_b(j):
                # HWDGE (sync) like the A stream: bulk traffic on the
                # gpsimd/SWDGE ring runs ~5x less efficient per byte and
                # starves the other rings at the SDMA engines
                c0, c1 = bblk[j]
                nc.sync.dma_start(bt[0:COUT, c0:c1], b_d[:, c0:c1])

            def copy_b(j, c0, c1):
                # bufB bottom half = bufA bottom half verbatim (same
                # partitions). DVE runs these at 4x (~0.7us); on ACT they
                # would wedge between PSUM evicts and back the PE up.
                nc.vector.tensor_copy(bt[64:128, c0:c1], at[64:128, c0:c1])

            # Keep A one block ahead of B: tile t's matmuls touch A-blocks
            # t and t+1 (rows l0..l0+5) but only B-block t. The first copy
            # is split at the A0/A1 boundary so tile 0's early chunks only
            # wait on A0.
            # Mask rides the sync ring in four 256KB pieces: as one 1MB
            # SWDGE (gpsimd) DMA its ~1.2us packets monopolize the SDMA
            # engines' round-robin and crawl the input loads.
            mt = cpool.tile([COUT, NPOS], U8, tag="mhigh")
            if need_zero_fix:
                mzt = cpool.tile([COUT, NPOS], U8, tag="mzero")
                zt = cpool.tile([COUT, TILE_POS], OUTDT, tag="zeros")
                nc.any.memset(zt[:], 0.0)

            def load_m(p):
                c0, c1 = p * (NPOS // 4), (p + 1) * (NPOS // 4)
                nc.sync.dma_start(mt[:, c0:c1], m_d[:, c0:c1])
                if need_zero_fix:
                    nc.sync.dma_start(mzt[:, c0:c1], mz_d[:, c0:c1])

            # Preload order is tuned so tile 0's first chunks only wait on
            # A0+B0: chunk 0-2 taps live in A-block 0, and the first half of
            # the bufB bottom-copy reads A0 alone.
            load_a(0)
            load_b(0)
            copy_b(0, bblk[0][0], 10 * WP)
            load_a(1)
            copy_b(0, 10 * WP, bblk[0][1])
            load_b(1)
            load_a(2)
            load_m(0)
            copy_b(1, *bblk[1])

            va = at[:].rearrange("p (r x) -> p r x", x=WP)
            vb = bt[:].rearrange("p (r x) -> p r x", x=WP)

            # Warm-up matmuls on dummy data while the first input blocks are
            # in flight: the PE HAM clock gate needs ~3.2us of sustained
            # activity to lift the 1.2GHz cold throttle (7 matmuls ramp it).
            dummy = cpool.tile([128, CHUNK], BF16, tag="dummy")
            nc.vector.memset(dummy[:], 0.0)
            warmp = pwpool.tile([128, CHUNK], F32, tag="warm")
            for _ in range(10):
                nc.tensor.matmul(
                    warmp[:], dummy[:, 0:128], dummy[:], start=True, stop=True
                )

            def mm_chunk(pv, l0):
                for c in range(3):
                    nc.tensor.matmul(
                        pv,
                        wt[:, c * 128:(c + 1) * 128],
                        va[:, l0:l0 + CHUNK_ROWS, c:c + W],
                        start=(c == 0),
                        stop=False,
                    )
                # tap (ky2,kx2) is K=64 but issued as K=128 with zeroed
                # weight rows 64-127: a K=64 LDWEIGHTS cannot use the
                # background weight slot and serializes against the
                # in-flight matmul
                nc.tensor.matmul(
                    pv,
                    wt[:, 4 * 128:5 * 128],
                    va[:, l0 + 2:l0 + 2 + CHUNK_ROWS, 2:2 + W],
                    start=False,
                    stop=False,
                )
                nc.tensor.matmul(
                    pv,
                    wt[:, 3 * 128:4 * 128],
                    vb[:, l0 + 1:l0 + 1 + CHUNK_ROWS, 1:1 + W],
                    start=False,
                    stop=True,
                )

            # Merge: ACT evicts each PSUM granule to SBUF bf16, a sync-ring
            # DMA moves the low half across partitions into the output
            # buffer, DVE overlays the high half per mask. ALL bulk DMA
            # (loads, moves, stores) shares the sync HWDGE ring: the SDMA
            # engines serve rings with strict priority (gpsimd > sync >
            # scalar), so anything on a lower ring starves while loads
            # stream; one FIFO gives every transfer a bounded, timely slot.
            # Each tile's HBM store is deferred into the NEXT tile's merge:
            # by then its predicate pass has finished, so the store issues
            # without a semaphore wait that would stall the sequencer. The
            # last tile merges per 512-col chunk (stores on the by-then-idle
            # scalar ring) so the post-matmul tail is one short chain.
            prev_store = None
            for t in range(N_TILES):
                if t + 3 < N_TILES:
                    load_a(t + 3)
                if t + 2 < N_TILES:
                    load_b(t + 2)
                    copy_b(t + 2, *bblk[t + 2])
                if 1 <= t <= 3:
                    load_m(t)
                out_sb = opool.tile([COUT, TILE_POS], OUTDT, tag="osb")
                last = t == N_TILES - 1
                ev = epool.tile([128, TILE_POS], OUTDT, tag="ev")
                for g in range(TILE_POS // GRAN):
                    pt = pspool.tile([128, GRAN], F32, tag="acc")
                    for cc in range(GRAN // CHUNK):
                        so = g * GRAN + cc * CHUNK
                        l0 = t * TROWS + so // W
                        pv = pt[:, cc * CHUNK:(cc + 1) * CHUNK].rearrange(
                            "p (r x) -> p r x", x=W
                        )
                        mm_chunk(pv, l0)
                        if last:
                            s = t * TILE_POS + so
                            nc.scalar.copy(
                                ev[:, so:so + CHUNK],
                                pt[:, cc * CHUNK:(cc + 1) * CHUNK],
                            )
                            nc.scalar.dma_start(
                                out_sb[:, so:so + CHUNK],
                                ev[64:128, so:so + CHUNK],
                            )
                            if so == 0 and prev_store is not None:
                                po, ps = prev_store
                                nc.sync.dma_start(
                                    out_d[:, ps:ps + TILE_POS], po[:]
                                )
                            nc.vector.copy_predicated(
                                out_sb[:, so:so + CHUNK], mt[:, s:s + CHUNK],
                                ev[0:64, so:so + CHUNK],
                            )
                            if need_zero_fix:
                                nc.vector.copy_predicated(
                                    out_sb[:, so:so + CHUNK],
                                    mzt[:, s:s + CHUNK], zt[:, 0:CHUNK],
                                )
                            nc.sync.dma_start(
                                out_d[:, s:s + CHUNK], out_sb[:, so:so + CHUNK]
                            )
                    if not last:
                        so = g * GRAN
                        s = t * TILE_POS + so
                        nc.scalar.copy(ev[:, so:so + GRAN], pt[:])
                        # while loads stream, the sync FIFO is the only ring
                        # that isn't starved, so early moves ride it; once
                        # loads finish (~tile 5) the scalar ring issues moves
                        # engine-ordered right behind its own evict
                        mv = nc.scalar if t >= 5 else nc.sync
                        mv.dma_start(
                            out_sb[:, so:so + GRAN], ev[64:128, so:so + GRAN]
                        )
                        if g == 0 and prev_store is not None:
                            po, ps = prev_store
                            nc.sync.dma_start(out_d[:, ps:ps + TILE_POS], po[:])
                        nc.vector.copy_predicated(
                            out_sb[:, so:so + GRAN], mt[:, s:s + GRAN],
                            ev[0:64, so:so + GRAN],
                        )
                        if need_zero_fix:
                            nc.vector.copy_predicated(
                                out_sb[:, so:so + GRAN], mzt[:, s:s + GRAN],
                                zt[:, 0:GRAN],
                            )
                if not last:
                    prev_store = (out_sb, t * TILE_POS)

    nc.compile()
    return nc


def _prepare_host(inx, mask_idx, inv_mask_idx, high_w, low1_w, low2_w):
    inx = np.asarray(inx, dtype=np.float32)
    mask_idx = np.asarray(mask_idx).astype(np.int64)
    inv_mask_idx = np.asarray(inv_mask_idx).astype(np.int64)
    high_w = np.asarray(high_w, dtype=np.float32)
    low1_w = np.asarray(low1_w, dtype=np.float32)
    low2_w = np.asarray(low2_w, dtype=np.float32)

    # zero-padded flat images I [B, 64, 130*130] bf16
    inxp = np.zeros((B, CIN, HP, WP), np.float32)
    inxp[:, :, 1:-1, 1:-1] = inx
    iflat = inxp.reshape(B, CIN, L).astype(ml_dtypes.bfloat16)

    # hbma: partitions 0-63 = I, 64-127 = I shifted one row (+130)
    hbma = np.zeros((B, 128, L), ml_dtypes.bfloat16)
    hbma[:, 0:64] = iflat
    hbma[:, 64:128, 0:L - WP] = iflat[:, :, WP:]
    # hbmb: I shifted +129 (bufB top half; bottom half is copied on-chip)
    hbmb = np.zeros((B, 64, L), ml_dtypes.bfloat16)
    hbmb[:, :, 0:L - (WP - 1)] = iflat[:, :, WP - 1:]

    # fold the low branch: W_low[o, c, ky, kx] = sum_m w2[o, m] w1[m, c, ky, kx]
    w2 = low2_w.reshape(COUT, -1).astype(np.float64)
    wl = np.einsum("om,mckl->ockl", w2, low1_w.astype(np.float64)).astype(np.float32)
    wh = high_w

    # weight blob [128, 5*128] bf16; lhsT[k, m], m = output col (0-63 high,
    # 64-127 low-folded); k partition halves match the buffer layouts above
    blob = np.zeros((128, WBLK), ml_dtypes.bfloat16)
    for c in range(3):
        blk = blob[:, c * 128:(c + 1) * 128]
        blk[0:64, 0:64] = wh[:, :, 0, c].T
        blk[0:64, 64:128] = wl[:, :, 0, c].T
        blk[64:128, 0:64] = wh[:, :, 1, c].T
        blk[64:128, 64:128] = wl[:, :, 1, c].T
    b3 = blob[:, 3 * 128:4 * 128]
    b3[0:64, 0:64] = wh[:, :, 2, 0].T
    b3[0:64, 64:128] = wl[:, :, 2, 0].T
    b3[64:128, 0:64] = wh[:, :, 2, 1].T
    b3[64:128, 64:128] = wl[:, :, 2, 1].T
    b4 = blob[:, 4 * 128:5 * 128]
    b4[0:64, 0:64] = wh[:, :, 2, 2].T
    b4[0:64, 64:128] = wl[:, :, 2, 2].T

    ntotal = B * NPOS
    in_mask = np.zeros(ntotal, dtype=bool)
    in_inv = np.zeros(ntotal, dtype=bool)
    in_mask[mask_idx] = True
    in_inv[inv_mask_idx] = True
    # high wins only where inv doesn't claim (reference scatters inv last)
    m_high = in_mask & ~in_inv
    neither = ~(in_mask | in_inv)
    need_zero_fix = bool(neither.any())

    in_maps = []
    for b in range(B):
        sl = slice(b * NPOS, (b + 1) * NPOS)
        mh = np.ascontiguousarray(
            np.broadcast_to(
                m_high[sl].astype(np.uint8)[None, :], (COUT, NPOS)
            )
        )
        m = {"hbma": hbma[b], "hbmb": hbmb[b], "wblob": blob, "mhigh": mh}
        if need_zero_fix:
            m["mzero"] = np.ascontiguousarray(
                np.broadcast_to(
                    neither[sl].astype(np.uint8)[None, :], (COUT, NPOS)
                )
            )
        in_maps.append(m)
    return in_maps, need_zero_fix


def _run(inputs: dict, trace: bool = False):
    in_maps, need_zero_fix = _prepare_host(**inputs)
    nc = _build_program(need_zero_fix)
    res = run_bass_kernel_spmd(nc, in_maps, list(range(B)), trace=trace)
    out = np.stack(
        [np.asarray(res.results[b]["out"]).astype(np.float32).reshape(COUT, H, W)
         for b in range(B)]
    ).astype(np.float32)
    return out, res


def kernel(**inputs) -> np.ndarray:
    out, _ = _run(inputs, trace=False)
    return out


# revision 31
# speedup vs baseline: 1.0182x; 1.0182x over previous
"""Trainium2 Bass kernel for nn_Conv_block_57690000720236.

Reference computation (per batch image b):
  - 3x3 SAME conv "high" branch: 64ch -> 64ch
  - low branch: 3x3 conv 64ch -> 16ch, then 1x1 conv 16ch -> 64ch
  - output position (b,y,x) takes the high value if its flat index is in
    mask_idx, the low value if in inv_mask_idx (inv wins on overlap), 0 if
    in neither.

Strategy (8 NeuronCores, data-parallel over batch):
  - Core b computes BOTH branches densely for image b; the low branch is
    folded on the host (W_low = w2 @ w1) so both branches are 3x3 convs,
    evaluated together as M=128 output columns (64 high + 64 low).
  - The padded image lives in SBUF as FLAT [row*130+col] buffers, so every
    conv tap is a column-offset view (no per-tile shifted copies):
      bufA: partitions 0-63 = I, 64-127 = I shifted one row (+130)
      bufB: partitions 0-63 = I+129,  64-127 = I+130
    5 matmul passes per 4-row chunk (K=576 padded to 5x128):
      pass 1-3: taps (ky0,c)+(ky1,c) from bufA view rows l0..l0+3, cols c
      pass 4:   tap (ky2,kx2) K=64 (zero-padded weights) from bufA rows
                l0+2..l0+5 cols 2
      pass 5:   taps (ky2,kx0)+(ky2,kx1) from bufB rows l0+1..l0+4 cols 1
  - Input DMA is 3 image copies (bufA both halves + bufB top from HBM);
    bufB's bottom half equals bufA's bottom half verbatim and is copied
    on-chip by DVE (same partitions), saving 2.2MB of HBM traffic.
  - Routing per 1024-col granule: ACT evicts PSUM to SBUF bf16, a DMA
    moves the low half across partitions into the output buffer, one DVE
    copy_predicated overlays the high half per mask. The SDMA engines
    serve DMA rings with strict priority (gpsimd > sync > scalar), so all
    bulk traffic shares the sync HWDGE FIFO while loads stream; the last
    tiles' moves shift to the scalar ring once loads finish, and the last
    tile merges per 512-col chunk so the post-matmul tail stays short.
"""

import numpy as np
import ml_dtypes

import concourse.bacc as bacc
import concourse.mybir as mybir
import concourse.tile as tile
from concourse.bass_utils import run_bass_kernel_spmd

B, CIN, H, W = 8, 64, 128, 128
COUT, KER = 64, 3
NPOS = H * W                 # 16384 positions per core
WP = W + 2                   # padded row length 130
HP = H + 2                   # padded rows 130
L = HP * WP                  # flat padded image length 16900
N_TILES = 8                  # image row-tiles
TROWS = H // N_TILES         # 16 output rows per tile
CHUNK_ROWS = 4               # output rows per matmul chunk
CHUNK = CHUNK_ROWS * W       # 512 positions per chunk
GRAN = 2 * CHUNK             # 1024 positions per merge granule
TILE_POS = TROWS * W         # 2048 positions per tile
BLK = TROWS * WP             # input block cols per tile (16 rows x 130)
F32 = mybir.dt.float32
BF16 = mybir.dt.bfloat16
U8 = mybir.dt.uint8
U16 = mybir.dt.uint16
OUTDT = mybir.dt.bfloat16    # on-chip merge + writeback dtype
WBLK = 5 * 128               # weight blob: 5 matmul blocks


def _build_program(need_zero_fix: bool):
    nc = bacc.Bacc("TRN2", target_bir_lowering=False, debug=False, num_devices=B)

    a_d = nc.dram_tensor("hbma", [128, L], BF16, kind="ExternalInput")
    b_d = nc.dram_tensor("hbmb", [COUT, L], BF16, kind="ExternalInput")
    w_d = nc.dram_tensor("wblob", [128, WBLK], BF16, kind="ExternalInput")
    m_d = nc.dram_tensor("mhigh", [COUT, NPOS], U8, kind="ExternalInput")
    if need_zero_fix:
        mz_d = nc.dram_tensor("mzero", [COUT, NPOS], U8, kind="ExternalInput")
    out_d = nc.dram_tensor("out", [COUT, NPOS], OUTDT, kind="ExternalOutput")

    # A-blocks: 16 image rows each, last block takes the 2 padded tail rows.
    ablk = [(j * BLK, min((j + 1) * BLK, L) if j < N_TILES - 1 else L)
            for j in range(N_TILES)]
    # B cols are only ever read for x in [131, 129*130); shift blocks by one
    # row so tile t's pass-5 window sits inside block t alone.
    bblk = [(j * BLK + WP, min((j + 1) * BLK + WP, 129 * WP))
            for j in range(N_TILES)]

    with tile.TileContext(nc) as tc:
        with (
            tc.tile_pool(name="const", bufs=1) as cpool,
            tc.tile_pool(name="outp", bufs=4) as opool,
            tc.tile_pool(name="evp", bufs=4) as epool,
            tc.tile_pool(name="psum", bufs=3, space="PSUM") as pspool,
            tc.tile_pool(name="psumw", bufs=1, space="PSUM") as pwpool,
        ):
            wt = cpool.tile([128, WBLK], BF16, tag="wblob")
            nc.sync.dma_start(wt[:], w_d[:])

            at = cpool.tile([128, L], BF16, tag="bufa")
            bt = cpool.tile([128, L], BF16, tag="bufb")

            def load_a(j):
                c0, c1 = ablk[j]
                nc.sync.dma_start(at[:, c0:c1], a_d[:, c0:c1])

            def load_b(j):
                # HWDGE (sync) like the A stream: bulk traffic on the
                # gpsimd/SWDGE ring runs ~5x less efficient per byte and
                # starves the other rings at the SDMA engines
                c0, c1 = bblk[j]
                nc.sync.dma_start(bt[0:COUT, c0:c1], b_d[:, c0:c1])

            def copy_b(j, c0, c1):
                # bufB bottom half = bufA bottom half verbatim (same
                # partitions). DVE runs these at 4x (~0.7us); on ACT they
                # would wedge between PSUM evicts and back the PE up.
                nc.vector.tensor_copy(bt[64:128, c0:c1], at[64:128, c0:c1])

            # Keep A one block ahead of B: tile t's matmuls touch A-blocks
            # t and t+1 (rows l0..l0+5) but only B-block t. The first copy
            # is split at the A0/A1 boundary so tile 0's early chunks only
            # wait on A0.
            # Mask rides the sync ring in four 256KB pieces: as one 1MB
            # SWDGE (gpsimd) DMA its ~1.2us packets monopolize the SDMA
            # engines' round-robin and crawl the input loads.
            mt = cpool.tile([COUT, NPOS], U8, tag="mhigh")
            if need_zero_fix:
                mzt = cpool.tile([COUT, NPOS], U8, tag="mzero")
                zt = cpool.tile([COUT, TILE_POS], OUTDT, tag="zeros")
                nc.any.memset(zt[:], 0.0)

            def load_m(p):
                c0, c1 = p * (NPOS // 4), (p + 1) * (NPOS // 4)
                nc.sync.dma_start(mt[:, c0:c1], m_d[:, c0:c1])
                if need_zero_fix:
                    nc.sync.dma_start(mzt[:, c0:c1], mz_d[:, c0:c1])

            # Preload order is tuned so tile 0's first chunks only wait on
            # A0+B0: chunk 0-2 taps live in A-block 0, and the first half of
            # the bufB bottom-copy reads A0 alone.
            load_a(0)
            load_b(0)
            copy_b(0, bblk[0][0], 10 * WP)
            load_a(1)
            copy_b(0, 10 * WP, bblk[0][1])
            load_b(1)
            load_a(2)
            load_m(0)
            copy_b(1, *bblk[1])

            va = at[:].rearrange("p (r x) -> p r x", x=WP)
            vb = bt[:].rearrange("p (r x) -> p r x", x=WP)

            # Warm-up matmuls on dummy data while the first input blocks are
            # in flight: the PE HAM clock gate needs ~3.2us of sustained
            # activity to lift the 1.2GHz cold throttle (7 matmuls ramp it).
            dummy = cpool.tile([128, CHUNK], BF16, tag="dummy")
            nc.vector.memset(dummy[:], 0.0)
            warmp = pwpool.tile([128, CHUNK], F32, tag="warm")
            for _ in range(10):
                nc.tensor.matmul(
                    warmp[:], dummy[:, 0:128], dummy[:], start=True, stop=True
                )

            def mm_chunk(pv, l0):
                for c in range(3):
                    nc.tensor.matmul(
                        pv,
                        wt[:, c * 128:(c + 1) * 128],
                        va[:, l0:l0 + CHUNK_ROWS, c:c + W],
                        start=(c == 0),
                        stop=False,
                    )
                # tap (ky2,kx2) is K=64 but issued as K=128 with zeroed
                # weight rows 64-127: a K=64 LDWEIGHTS cannot use the
                # background weight slot and serializes against the
                # in-flight matmul
                nc.tensor.matmul(
                    pv,
                    wt[:, 4 * 128:5 * 128],
                    va[:, l0 + 2:l0 + 2 + CHUNK_ROWS, 2:2 + W],
                    start=False,
                    stop=False,
                )
                nc.tensor.matmul(
                    pv,
                    wt[:, 3 * 128:4 * 128],
                    vb[:, l0 + 1:l0 + 1 + CHUNK_ROWS, 1:1 + W],
                    start=False,
                    stop=True,
                )

            # Merge: ACT evicts each PSUM granule to SBUF bf16, a sync-ring
            # DMA moves the low half across partitions into the output
            # buffer, DVE overlays the high half per mask. ALL bulk DMA
            # (loads, moves, stores) shares the sync HWDGE ring: the SDMA
            # engines serve rings with strict priority (gpsimd > sync >
            # scalar), so anything on a lower ring starves while loads
            # stream; one FIFO gives every transfer a bounded, timely slot.
            # Each tile's HBM store is deferred into the NEXT tile's merge:
            # by then its predicate pass has finished, so the store issues
            # without a semaphore wait that would stall the sequencer. The
            # last tile merges per 512-col chunk (stores on the by-then-idle
            # scalar ring) so the post-matmul tail is one short chain.
            prev_store = None
            for t in range(N_TILES):
                if t + 3 < N_TILES:
                    load_a(t + 3)
                if t + 2 < N_TILES:
                    load_b(t + 2)
                    copy_b(t + 2, *bblk[t + 2])
                if 1 <= t <= 3:
                    load_m(t)
                out_sb = opool.tile([COUT, TILE_POS], OUTDT, tag="osb")
                last = t == N_TILES - 1
                ev = epool.tile([128, TILE_POS], OUTDT, tag="ev")
                for g in range(TILE_POS // GRAN):
                    pt = pspool.tile([128, GRAN], F32, tag="acc")
                    for cc in range(GRAN // CHUNK):
                        so = g * GRAN + cc * CHUNK
                        l0 = t * TROWS + so // W
                        pv = pt[:, cc * CHUNK:(cc + 1) * CHUNK].rearrange(
                            "p (r x) -> p r x", x=W
                        )
                        mm_chunk(pv, l0)
                        if last:
                            s = t * TILE_POS + so
                            nc.scalar.copy(
                                ev[:, so:so + CHUNK],
                                pt[:, cc * CHUNK:(cc + 1) * CHUNK],
                            )
                            nc.sync.dma_start(
                                out_sb[:, so:so + CHUNK],
                                ev[64:128, so:so + CHUNK],
                            )
                            if so == 0 and prev_store is not None:
                                po, ps = prev_store
                                nc.sync.dma_start(
                                    out_d[:, ps:ps + TILE_POS], po[:]
                                )
                            nc.vector.copy_predicated(
                                out_sb[:, so:so + CHUNK], mt[:, s:s + CHUNK],
                                ev[0:64, so:so + CHUNK],
                            )
                            if need_zero_fix:
                                nc.vector.copy_predicated(
                                    out_sb[:, so:so + CHUNK],
                                    mzt[:, s:s + CHUNK], zt[:, 0:CHUNK],
                                )
                            nc.sync.dma_start(
                                out_d[:, s:s + CHUNK], out_sb[:, so:so + CHUNK]
                            )
                    if not last:
                        so = g * GRAN
                        s = t * TILE_POS + so
                        nc.scalar.copy(ev[:, so:so + GRAN], pt[:])
                        # the sync FIFO is the only ring that isn't starved
                        # while it has queued bytes; a move on the scalar
                        # ring can fill that ring's FIFO and backpressure
                        # the ACT sequencer (stalling evicts, then the PE)
                        nc.sync.dma_start(
                            out_sb[:, so:so + GRAN], ev[64:128, so:so + GRAN]
                        )
                        if g == 0 and prev_store is not None:
                            po, ps = prev_store
                            nc.sync.dma_start(out_d[:, ps:ps + TILE_POS], po[:])
                        nc.vector.copy_predicated(
                            out_sb[:, so:so + GRAN], mt[:, s:s + GRAN],
                            ev[0:64, so:so + GRAN],
                        )
                        if need_zero_fix:
                            nc.vector.copy_predicated(
                                out_sb[:, so:so + GRAN], mzt[:, s:s + GRAN],
                                zt[:, 0:GRAN],
                            )
                if not last:
                    prev_store = (out_sb, t * TILE_POS)

    nc.compile()
    return nc


def _prepare_host(inx, mask_idx, inv_mask_idx, high_w, low1_w, low2_w):
    inx = np.asarray(inx, dtype=np.float32)
    mask_idx = np.asarray(mask_idx).astype(np.int64)
    inv_mask_idx = np.asarray(inv_mask_idx).astype(np.int64)
    high_w = np.asarray(high_w, dtype=np.float32)
    low1_w = np.asarray(low1_w, dtype=np.float32)
    low2_w = np.asarray(low2_w, dtype=np.float32)

    # zero-padded flat images I [B, 64, 130*130] bf16
    inxp = np.zeros((B, CIN, HP, WP), np.float32)
    inxp[:, :, 1:-1, 1:-1] = inx
    iflat = inxp.reshape(B, CIN, L).astype(ml_dtypes.bfloat16)

    # hbma: partitions 0-63 = I, 64-127 = I shifted one row (+130)
    hbma = np.zeros((B, 128, L), ml_dtypes.bfloat16)
    hbma[:, 0:64] = iflat
    hbma[:, 64:128, 0:L - WP] = iflat[:, :, WP:]
    # hbmb: I shifted +129 (bufB top half; bottom half is copied on-chip)
    hbmb = np.zeros((B, 64, L), ml_dtypes.bfloat16)
    hbmb[:, :, 0:L - (WP - 1)] = iflat[:, :, WP - 1:]

    # fold the low branch: W_low[o, c, ky, kx] = sum_m w2[o, m] w1[m, c, ky, kx]
    w2 = low2_w.reshape(COUT, -1).astype(np.float64)
    wl = np.einsum("om,mckl->ockl", w2, low1_w.astype(np.float64)).astype(np.float32)
    wh = high_w

    # weight blob [128, 5*128] bf16; lhsT[k, m], m = output col (0-63 high,
    # 64-127 low-folded); k partition halves match the buffer layouts above
    blob = np.zeros((128, WBLK), ml_dtypes.bfloat16)
    for c in range(3):
        blk = blob[:, c * 128:(c + 1) * 128]
        blk[0:64, 0:64] = wh[:, :, 0, c].T
        blk[0:64, 64:128] = wl[:, :, 0, c].T
        blk[64:128, 0:64] = wh[:, :, 1, c].T
        blk[64:128, 64:128] = wl[:, :, 1, c].T
    b3 = blob[:, 3 * 128:4 * 128]
    b3[0:64, 0:64] = wh[:, :, 2, 0].T
    b3[0:64, 64:128] = wl[:, :, 2, 0].T
    b3[64:128, 0:64] = wh[:, :, 2, 1].T
    b3[64:128, 64:128] = wl[:, :, 2, 1].T
    b4 = blob[:, 4 * 128:5 * 128]
    b4[0:64, 0:64] = wh[:, :, 2, 2].T
    b4[0:64, 64:128] = wl[:, :, 2, 2].T

    ntotal = B * NPOS
    in_mask = np.zeros(ntotal, dtype=bool)
    in_inv = np.zeros(ntotal, dtype=bool)
    in_mask[mask_idx] = True
    in_inv[inv_mask_idx] = True
    # high wins only where inv doesn't claim (reference scatters inv last)
    m_high = in_mask & ~in_inv
    neither = ~(in_mask | in_inv)
    need_zero_fix = bool(neither.any())

    in_maps = []
    for b in range(B):
        sl = slice(b * NPOS, (b + 1) * NPOS)
        mh = np.ascontiguousarray(
            np.broadcast_to(
                m_high[sl].astype(np.uint8)[None, :], (COUT, NPOS)
            )
        )
        m = {"hbma": hbma[b], "hbmb": hbmb[b], "wblob": blob, "mhigh": mh}
        if need_zero_fix:
            m["mzero"] = np.ascontiguousarray(
                np.broadcast_to(
                    neither[sl].astype(np.uint8)[None, :], (COUT, NPOS)
                )
            )
        in_maps.append(m)
    return in_maps, need_zero_fix


def _run(inputs: dict, trace: bool = False):
    in_maps, need_zero_fix = _prepare_host(**inputs)
    nc = _build_program(need_zero_fix)
    res = run_bass_kernel_spmd(nc, in_maps, list(range(B)), trace=trace)
    out = np.stack(
        [np.asarray(res.results[b]["out"]).astype(np.float32).reshape(COUT, H, W)
         for b in range(B)]
    ).astype(np.float32)
    return out, res


def kernel(**inputs) -> np.ndarray:
    out, _ = _run(inputs, trace=False)
    return out


# revision 32
# speedup vs baseline: 1.0256x; 1.0073x over previous
"""Trainium2 Bass kernel for nn_Conv_block_57690000720236.

Reference computation (per batch image b):
  - 3x3 SAME conv "high" branch: 64ch -> 64ch
  - low branch: 3x3 conv 64ch -> 16ch, then 1x1 conv 16ch -> 64ch
  - output position (b,y,x) takes the high value if its flat index is in
    mask_idx, the low value if in inv_mask_idx (inv wins on overlap), 0 if
    in neither.

Strategy (8 NeuronCores, data-parallel over batch):
  - Core b computes BOTH branches densely for image b; the low branch is
    folded on the host (W_low = w2 @ w1) so both branches are 3x3 convs,
    evaluated together as M=128 output columns (64 high + 64 low).
  - The padded image lives in SBUF as FLAT [row*130+col] buffers, so every
    conv tap is a column-offset view (no per-tile shifted copies):
      bufA: partitions 0-63 = I, 64-127 = I shifted one row (+130)
      bufB: partitions 0-63 = I+129,  64-127 = I+130
    5 matmul passes per 4-row chunk (K=576 padded to 5x128):
      pass 1-3: taps (ky0,c)+(ky1,c) from bufA view rows l0..l0+3, cols c
      pass 4:   tap (ky2,kx2) K=64 (zero-padded weights) from bufA rows
                l0+2..l0+5 cols 2
      pass 5:   taps (ky2,kx0)+(ky2,kx1) from bufB rows l0+1..l0+4 cols 1
  - Input DMA is 3 image copies (bufA both halves + bufB top from HBM);
    bufB's bottom half equals bufA's bottom half verbatim and is copied
    on-chip by DVE (same partitions), saving 2.2MB of HBM traffic.
  - Routing per 1024-col granule: ACT evicts PSUM to SBUF bf16, a DMA
    moves the low half across partitions into the output buffer, one DVE
    copy_predicated overlays the high half per mask. The SDMA engines
    serve DMA rings with strict priority (gpsimd > sync > scalar), so all
    bulk traffic shares the sync HWDGE FIFO while loads stream; the last
    tiles' moves shift to the scalar ring once loads finish, and the last
    tile merges per 512-col chunk so the post-matmul tail stays short.
"""

import numpy as np
import ml_dtypes

import concourse.bacc as bacc
import concourse.mybir as mybir
import concourse.tile as tile
from concourse.bass_utils import run_bass_kernel_spmd

B, CIN, H, W = 8, 64, 128, 128
COUT, KER = 64, 3
NPOS = H * W                 # 16384 positions per core
WP = W + 2                   # padded row length 130
HP = H + 2                   # padded rows 130
L = HP * WP                  # flat padded image length 16900
N_TILES = 8                  # image row-tiles
TROWS = H // N_TILES         # 16 output rows per tile
CHUNK_ROWS = 4               # output rows per matmul chunk
CHUNK = CHUNK_ROWS * W       # 512 positions per chunk
GRAN = 2 * CHUNK             # 1024 positions per merge granule
TILE_POS = TROWS * W         # 2048 positions per tile
BLK = TROWS * WP             # input block cols per tile (16 rows x 130)
F32 = mybir.dt.float32
BF16 = mybir.dt.bfloat16
U8 = mybir.dt.uint8
U16 = mybir.dt.uint16
OUTDT = mybir.dt.bfloat16    # on-chip merge + writeback dtype
WBLK = 5 * 128               # weight blob: 5 matmul blocks


def _build_program(need_zero_fix: bool):
    nc = bacc.Bacc("TRN2", target_bir_lowering=False, debug=False, num_devices=B)

    a_d = nc.dram_tensor("hbma", [128, L], BF16, kind="ExternalInput")
    b_d = nc.dram_tensor("hbmb", [COUT, L], BF16, kind="ExternalInput")
    w_d = nc.dram_tensor("wblob", [128, WBLK], BF16, kind="ExternalInput")
    m_d = nc.dram_tensor("mhigh", [COUT, NPOS], U8, kind="ExternalInput")
    if need_zero_fix:
        mz_d = nc.dram_tensor("mzero", [COUT, NPOS], U8, kind="ExternalInput")
    out_d = nc.dram_tensor("out", [COUT, NPOS], OUTDT, kind="ExternalOutput")

    # A-blocks: 16 image rows each, last block takes the 2 padded tail rows.
    ablk = [(j * BLK, min((j + 1) * BLK, L) if j < N_TILES - 1 else L)
            for j in range(N_TILES)]
    # B cols are only ever read for x in [131, 129*130); shift blocks by one
    # row so tile t's pass-5 window sits inside block t alone.
    bblk = [(j * BLK + WP, min((j + 1) * BLK + WP, 129 * WP))
            for j in range(N_TILES)]

    with tile.TileContext(nc) as tc:
        with (
            tc.tile_pool(name="const", bufs=1) as cpool,
            tc.tile_pool(name="outp", bufs=4) as opool,
            tc.tile_pool(name="evp", bufs=4) as epool,
            tc.tile_pool(name="psum", bufs=3, space="PSUM") as pspool,
            tc.tile_pool(name="psumw", bufs=1, space="PSUM") as pwpool,
        ):
            wt = cpool.tile([128, WBLK], BF16, tag="wblob")
            nc.sync.dma_start(wt[:], w_d[:])

            at = cpool.tile([128, L], BF16, tag="bufa")
            bt = cpool.tile([128, L], BF16, tag="bufb")

            def load_a(j):
                c0, c1 = ablk[j]
                nc.sync.dma_start(at[:, c0:c1], a_d[:, c0:c1])

            def load_b(j):
                # HWDGE (sync) like the A stream: bulk traffic on the
                # gpsimd/SWDGE ring runs ~5x less efficient per byte and
                # starves the other rings at the SDMA engines
                c0, c1 = bblk[j]
                nc.sync.dma_start(bt[0:COUT, c0:c1], b_d[:, c0:c1])

            def copy_b(j, c0, c1):
                # bufB bottom half = bufA bottom half verbatim (same
                # partitions). DVE runs these at 4x (~0.7us); on ACT they
                # would wedge between PSUM evicts and back the PE up.
                nc.vector.tensor_copy(bt[64:128, c0:c1], at[64:128, c0:c1])

            # Keep A one block ahead of B: tile t's matmuls touch A-blocks
            # t and t+1 (rows l0..l0+5) but only B-block t. The first copy
            # is split at the A0/A1 boundary so tile 0's early chunks only
            # wait on A0.
            # Mask rides the sync ring in four 256KB pieces: as one 1MB
            # SWDGE (gpsimd) DMA its ~1.2us packets monopolize the SDMA
            # engines' round-robin and crawl the input loads.
            mt = cpool.tile([COUT, NPOS], U8, tag="mhigh")
            if need_zero_fix:
                mzt = cpool.tile([COUT, NPOS], U8, tag="mzero")
                zt = cpool.tile([COUT, TILE_POS], OUTDT, tag="zeros")
                nc.any.memset(zt[:], 0.0)

            def load_m(p):
                c0, c1 = p * (NPOS // 4), (p + 1) * (NPOS // 4)
                nc.sync.dma_start(mt[:, c0:c1], m_d[:, c0:c1])
                if need_zero_fix:
                    nc.sync.dma_start(mzt[:, c0:c1], mz_d[:, c0:c1])

            # Preload order is tuned so tile 0's first chunks only wait on
            # A0+B0: chunk 0-2 taps live in A-block 0, and the first half of
            # the bufB bottom-copy reads A0 alone.
            load_a(0)
            load_b(0)
            copy_b(0, bblk[0][0], 10 * WP)
            load_a(1)
            copy_b(0, 10 * WP, bblk[0][1])
            load_b(1)
            load_a(2)
            load_m(0)
            copy_b(1, *bblk[1])

            va = at[:].rearrange("p (r x) -> p r x", x=WP)
            vb = bt[:].rearrange("p (r x) -> p r x", x=WP)

            # Warm-up matmuls on dummy data while the first input blocks are
            # in flight: the PE HAM clock gate needs ~3.2us of sustained
            # activity to lift the 1.2GHz cold throttle (7 matmuls ramp it).
            dummy = cpool.tile([128, CHUNK], BF16, tag="dummy")
            nc.vector.memset(dummy[:], 0.0)
            warmp = pwpool.tile([128, CHUNK], F32, tag="warm")
            for _ in range(10):
                nc.tensor.matmul(
                    warmp[:], dummy[:, 0:128], dummy[:], start=True, stop=True
                )

            def mm_chunk(pv, l0):
                for c in range(3):
                    nc.tensor.matmul(
                        pv,
                        wt[:, c * 128:(c + 1) * 128],
                        va[:, l0:l0 + CHUNK_ROWS, c:c + W],
                        start=(c == 0),
                        stop=False,
                    )
                # tap (ky2,kx2) is K=64 but issued as K=128 with zeroed
                # weight rows 64-127: a K=64 LDWEIGHTS cannot use the
                # background weight slot and serializes against the
                # in-flight matmul
                nc.tensor.matmul(
                    pv,
                    wt[:, 4 * 128:5 * 128],
                    va[:, l0 + 2:l0 + 2 + CHUNK_ROWS, 2:2 + W],
                    start=False,
                    stop=False,
                )
                nc.tensor.matmul(
                    pv,
                    wt[:, 3 * 128:4 * 128],
                    vb[:, l0 + 1:l0 + 1 + CHUNK_ROWS, 1:1 + W],
                    start=False,
                    stop=True,
                )

            # Merge: ACT evicts each PSUM granule to SBUF bf16, a sync-ring
            # DMA moves the low half across partitions into the output
            # buffer, DVE overlays the high half per mask. ALL bulk DMA
            # (loads, moves, stores) shares the sync HWDGE ring: the SDMA
            # engines serve rings with strict priority (gpsimd > sync >
            # scalar), so anything on a lower ring starves while loads
            # stream; one FIFO gives every transfer a bounded, timely slot.
            # Each tile's HBM store is deferred into the NEXT tile's merge:
            # by then its predicate pass has finished, so the store issues
            # without a semaphore wait that would stall the sequencer. The
            # last tile merges per 512-col chunk (stores on the by-then-idle
            # scalar ring) so the post-matmul tail is one short chain.
            prev_store = None
            for t in range(N_TILES):
                if t + 3 < N_TILES:
                    load_a(t + 3)
                if t + 2 < N_TILES:
                    load_b(t + 2)
                    copy_b(t + 2, *bblk[t + 2])
                if 1 <= t <= 3:
                    load_m(t)
                out_sb = opool.tile([COUT, TILE_POS], OUTDT, tag="osb")
                last = t == N_TILES - 1
                ev = epool.tile([128, TILE_POS], OUTDT, tag="ev")
                for g in range(TILE_POS // GRAN):
                    pt = pspool.tile([128, GRAN], F32, tag="acc")
                    for cc in range(GRAN // CHUNK):
                        so = g * GRAN + cc * CHUNK
                        l0 = t * TROWS + so // W
                        pv = pt[:, cc * CHUNK:(cc + 1) * CHUNK].rearrange(
                            "p (r x) -> p r x", x=W
                        )
                        mm_chunk(pv, l0)
                        if last:
                            s = t * TILE_POS + so
                            nc.scalar.copy(
                                ev[:, so:so + CHUNK],
                                pt[:, cc * CHUNK:(cc + 1) * CHUNK],
                            )
                            nc.scalar.dma_start(
                                out_sb[:, so:so + CHUNK],
                                ev[64:128, so:so + CHUNK],
                            )
                            if so == 0 and prev_store is not None:
                                po, ps = prev_store
                                nc.sync.dma_start(
                                    out_d[:, ps:ps + TILE_POS], po[:]
                                )
                            nc.vector.copy_predicated(
                                out_sb[:, so:so + CHUNK], mt[:, s:s + CHUNK],
                                ev[0:64, so:so + CHUNK],
                            )
                            if need_zero_fix:
                                nc.vector.copy_predicated(
                                    out_sb[:, so:so + CHUNK],
                                    mzt[:, s:s + CHUNK], zt[:, 0:CHUNK],
                                )
                            nc.sync.dma_start(
                                out_d[:, s:s + CHUNK], out_sb[:, so:so + CHUNK]
                            )
                    if not last:
                        so = g * GRAN
                        s = t * TILE_POS + so
                        nc.scalar.copy(ev[:, so:so + GRAN], pt[:])
                        # while loads stream, the sync FIFO is the only ring
                        # that isn't starved, so early moves ride it; once
                        # loads finish (~tile 5) the scalar ring issues moves
                        # engine-ordered right behind its own evict
                        mv = nc.scalar if t >= 5 else nc.sync
                        mv.dma_start(
                            out_sb[:, so:so + GRAN], ev[64:128, so:so + GRAN]
                        )
                        if g == 0 and prev_store is not None:
                            po, ps = prev_store
                            nc.sync.dma_start(out_d[:, ps:ps + TILE_POS], po[:])
                        nc.vector.copy_predicated(
                            out_sb[:, so:so + GRAN], mt[:, s:s + GRAN],
                            ev[0:64, so:so + GRAN],
                        )
                        if need_zero_fix:
                            nc.vector.copy_predicated(
                                out_sb[:, so:so + GRAN], mzt[:, s:s + GRAN],
                                zt[:, 0:GRAN],
                            )
                if not last:
                    prev_store = (out_sb, t * TILE_POS)

    nc.compile()
    return nc


def _prepare_host(inx, mask_idx, inv_mask_idx, high_w, low1_w, low2_w):
    inx = np.asarray(inx, dtype=np.float32)
    mask_idx = np.asarray(mask_idx).astype(np.int64)
    inv_mask_idx = np.asarray(inv_mask_idx).astype(np.int64)
    high_w = np.asarray(high_w, dtype=np.float32)
    low1_w = np.asarray(low1_w, dtype=np.float32)
    low2_w = np.asarray(low2_w, dtype=np.float32)

    # zero-padded flat images I [B, 64, 130*130] bf16
    inxp = np.zeros((B, CIN, HP, WP), np.float32)
    inxp[:, :, 1:-1, 1:-1] = inx
    iflat = inxp.reshape(B, CIN, L).astype(ml_dtypes.bfloat16)

    # hbma: partitions 0-63 = I, 64-127 = I shifted one row (+130)
    hbma = np.zeros((B, 128, L), ml_dtypes.bfloat16)
    hbma[:, 0:64] = iflat
    hbma[:, 64:128, 0:L - WP] = iflat[:, :, WP:]
    # hbmb: I shifted +129 (bufB top half; bottom half is copied on-chip)
    hbmb = np.zeros((B, 64, L), ml_dtypes.bfloat16)
    hbmb[:, :, 0:L - (WP - 1)] = iflat[:, :, WP - 1:]

    # fold the low branch: W_low[o, c, ky, kx] = sum_m w2[o, m] w1[m, c, ky, kx]
    w2 = low2_w.reshape(COUT, -1).astype(np.float64)
    wl = np.einsum("om,mckl->ockl", w2, low1_w.astype(np.float64)).astype(np.float32)
    wh = high_w

    # weight blob [128, 5*128] bf16; lhsT[k, m], m = output col (0-63 high,
    # 64-127 low-folded); k partition halves match the buffer layouts above
    blob = np.zeros((128, WBLK), ml_dtypes.bfloat16)
    for c in range(3):
        blk = blob[:, c * 128:(c + 1) * 128]
        blk[0:64, 0:64] = wh[:, :, 0, c].T
        blk[0:64, 64:128] = wl[:, :, 0, c].T
        blk[64:128, 0:64] = wh[:, :, 1, c].T
        blk[64:128, 64:128] = wl[:, :, 1, c].T
    b3 = blob[:, 3 * 128:4 * 128]
    b3[0:64, 0:64] = wh[:, :, 2, 0].T
    b3[0:64, 64:128] = wl[:, :, 2, 0].T
    b3[64:128, 0:64] = wh[:, :, 2, 1].T
    b3[64:128, 64:128] = wl[:, :, 2, 1].T
    b4 = blob[:, 4 * 128:5 * 128]
    b4[0:64, 0:64] = wh[:, :, 2, 2].T
    b4[0:64, 64:128] = wl[:, :, 2, 2].T

    ntotal = B * NPOS
    in_mask = np.zeros(ntotal, dtype=bool)
    in_inv = np.zeros(ntotal, dtype=bool)
    in_mask[mask_idx] = True
    in_inv[inv_mask_idx] = True
    # high wins only where inv doesn't claim (reference scatters inv last)
    m_high = in_mask & ~in_inv
    neither = ~(in_mask | in_inv)
    need_zero_fix = bool(neither.any())

    in_maps = []
    for b in range(B):
        sl = slice(b * NPOS, (b + 1) * NPOS)
        mh = np.ascontiguousarray(
            np.broadcast_to(
                m_high[sl].astype(np.uint8)[None, :], (COUT, NPOS)
            )
        )
        m = {"hbma": hbma[b], "hbmb": hbmb[b], "wblob": blob, "mhigh": mh}
        if need_zero_fix:
            m["mzero"] = np.ascontiguousarray(
                np.broadcast_to(
                    neither[sl].astype(np.uint8)[None, :], (COUT, NPOS)
                )
            )
        in_maps.append(m)
    return in_maps, need_zero_fix


def _run(inputs: dict, trace: bool = False):
    in_maps, need_zero_fix = _prepare_host(**inputs)
    nc = _build_program(need_zero_fix)
    res = run_bass_kernel_spmd(nc, in_maps, list(range(B)), trace=trace)
    out = np.stack(
        [np.asarray(res.results[b]["out"]).astype(np.float32).reshape(COUT, H, W)
         for b in range(B)]
    ).astype(np.float32)
    return out, res


def kernel(**inputs) -> np.ndarray:
    out, _ = _run(inputs, trace=False)
    return out


# revision 34
# speedup vs baseline: 1.0489x; 1.0227x over previous
"""Trainium2 Bass kernel for nn_Conv_block_57690000720236.

Reference computation (per batch image b):
  - 3x3 SAME conv "high" branch: 64ch -> 64ch
  - low branch: 3x3 conv 64ch -> 16ch, then 1x1 conv 16ch -> 64ch
  - output position (b,y,x) takes the high value if its flat index is in
    mask_idx, the low value if in inv_mask_idx (inv wins on overlap), 0 if
    in neither.

Strategy (8 NeuronCores, data-parallel over batch):
  - Core b computes BOTH branches densely for image b; the low branch is
    folded on the host (W_low = w2 @ w1) so both branches are 3x3 convs,
    evaluated together as M=128 output columns (64 high + 64 low).
  - The padded image lives in SBUF as FLAT [row*130+col] buffers, so every
    conv tap is a column-offset view (no per-tile shifted copies):
      bufA: partitions 0-63 = I, 64-127 = I shifted one row (+130)
      bufB: partitions 0-63 = I+129,  64-127 = I+130
    5 matmul passes per 4-row chunk (K=576 padded to 5x128):
      pass 1-3: taps (ky0,c)+(ky1,c) from bufA view rows l0..l0+3, cols c
      pass 4:   tap (ky2,kx2) K=64 (zero-padded weights) from bufA rows
                l0+2..l0+5 cols 2
      pass 5:   taps (ky2,kx0)+(ky2,kx1) from bufB rows l0+1..l0+4 cols 1
  - Input DMA is 3 image copies (bufA both halves + bufB top from HBM);
    bufB's bottom half equals bufA's bottom half verbatim and is copied
    on-chip by ACT/DVE (same partitions), saving 2.2MB of HBM traffic.
  - Routing per 1024-col granule: ACT evicts PSUM to SBUF bf16, a gpsimd
    (SWDGE) DMA moves the low half across partitions into the output
    buffer, one DVE copy_predicated overlays the high half per mask.
    Moves ride the otherwise-idle gpsimd queue so they never sit behind
    the input loads (sync) or the output stores (scalar).
"""

import numpy as np
import ml_dtypes

import concourse.bacc as bacc
import concourse.mybir as mybir
import concourse.tile as tile
from concourse.bass_utils import run_bass_kernel_spmd

B, CIN, H, W = 8, 64, 128, 128
COUT, KER = 64, 3
NPOS = H * W                 # 16384 positions per core
WP = W + 2                   # padded row length 130
HP = H + 2                   # padded rows 130
L = HP * WP                  # flat padded image length 16900
N_TILES = 8                  # image row-tiles
TROWS = H // N_TILES         # 16 output rows per tile
CHUNK_ROWS = 4               # output rows per matmul chunk
CHUNK = CHUNK_ROWS * W       # 512 positions per chunk
GRAN = 2 * CHUNK             # 1024 positions per merge granule
TILE_POS = TROWS * W         # 2048 positions per tile
BLK = TROWS * WP             # input block cols per tile (16 rows x 130)
F32 = mybir.dt.float32
BF16 = mybir.dt.bfloat16
U8 = mybir.dt.uint8
U16 = mybir.dt.uint16
OUTDT = mybir.dt.bfloat16    # on-chip merge + writeback dtype
WBLK = 5 * 128               # weight blob: 5 matmul blocks


def _build_program(need_zero_fix: bool):
    nc = bacc.Bacc("TRN2", target_bir_lowering=False, debug=False, num_devices=B)

    a_d = nc.dram_tensor("hbma", [128, L], BF16, kind="ExternalInput")
    b_d = nc.dram_tensor("hbmb", [COUT, L], BF16, kind="ExternalInput")
    w_d = nc.dram_tensor("wblob", [128, WBLK], BF16, kind="ExternalInput")
    m_d = nc.dram_tensor("mhigh", [COUT, NPOS], U8, kind="ExternalInput")
    if need_zero_fix:
        mz_d = nc.dram_tensor("mzero", [COUT, NPOS], U8, kind="ExternalInput")
    out_d = nc.dram_tensor("out", [COUT, NPOS], OUTDT, kind="ExternalOutput")

    # A-blocks: 16 image rows each, last block takes the 2 padded tail rows.
    ablk = [(j * BLK, min((j + 1) * BLK, L) if j < N_TILES - 1 else L)
            for j in range(N_TILES)]
    # B cols are only ever read for x in [131, 129*130); shift blocks by one
    # row so tile t's pass-5 window sits inside block t alone.
    bblk = [(j * BLK + WP, min((j + 1) * BLK + WP, 129 * WP))
            for j in range(N_TILES)]

    with tile.TileContext(nc) as tc:
        with (
            tc.tile_pool(name="const", bufs=1) as cpool,
            tc.tile_pool(name="outp", bufs=4) as opool,
            tc.tile_pool(name="evp", bufs=4) as epool,
            tc.tile_pool(name="psum", bufs=3, space="PSUM") as pspool,
            tc.tile_pool(name="psumw", bufs=1, space="PSUM") as pwpool,
        ):
            wt = cpool.tile([128, WBLK], BF16, tag="wblob")
            nc.sync.dma_start(wt[:], w_d[:])

            at = cpool.tile([128, L], BF16, tag="bufa")
            bt = cpool.tile([128, L], BF16, tag="bufb")

            def load_a(j):
                c0, c1 = ablk[j]
                nc.sync.dma_start(at[:, c0:c1], a_d[:, c0:c1])

            def load_b(j):
                # HWDGE (sync) like the A stream: bulk traffic on the
                # gpsimd/SWDGE ring runs ~5x less efficient per byte and
                # starves the other rings at the SDMA engines
                c0, c1 = bblk[j]
                nc.sync.dma_start(bt[0:COUT, c0:c1], b_d[:, c0:c1])

            def copy_b(j, c0, c1):
                # bufB bottom half = bufA bottom half verbatim (same
                # partitions). DVE runs these at 4x (~0.7us); on ACT they
                # would wedge between PSUM evicts and back the PE up.
                nc.vector.tensor_copy(bt[64:128, c0:c1], at[64:128, c0:c1])

            # Keep A one block ahead of B: tile t's matmuls touch A-blocks
            # t and t+1 (rows l0..l0+5) but only B-block t. The first copy
            # is split at the A0/A1 boundary so tile 0's early chunks only
            # wait on A0.
            # Mask rides the sync ring in four 256KB pieces: as one 1MB
            # SWDGE (gpsimd) DMA its ~1.2us packets monopolize the SDMA
            # engines' round-robin and crawl the input loads.
            mt = cpool.tile([COUT, NPOS], U8, tag="mhigh")
            if need_zero_fix:
                mzt = cpool.tile([COUT, NPOS], U8, tag="mzero")
                zt = cpool.tile([COUT, TILE_POS], OUTDT, tag="zeros")
                nc.any.memset(zt[:], 0.0)

            def load_m(p):
                c0, c1 = p * (NPOS // 4), (p + 1) * (NPOS // 4)
                nc.sync.dma_start(mt[:, c0:c1], m_d[:, c0:c1])
                if need_zero_fix:
                    nc.sync.dma_start(mzt[:, c0:c1], mz_d[:, c0:c1])

            # Preload order is tuned so tile 0's first chunks only wait on
            # A0+B0: chunk 0-2 taps live in A-block 0, and the first half of
            # the bufB bottom-copy reads A0 alone.
            load_a(0)
            load_b(0)
            copy_b(0, bblk[0][0], 10 * WP)
            load_a(1)
            copy_b(0, 10 * WP, bblk[0][1])
            load_b(1)
            load_a(2)
            load_m(0)
            copy_b(1, *bblk[1])

            va = at[:].rearrange("p (r x) -> p r x", x=WP)
            vb = bt[:].rearrange("p (r x) -> p r x", x=WP)

            # Warm-up matmuls on dummy data while the first input blocks are
            # in flight: the PE HAM clock gate needs ~3.2us of sustained
            # activity to lift the 1.2GHz cold throttle (7 matmuls ramp it).
            dummy = cpool.tile([128, CHUNK], BF16, tag="dummy")
            nc.vector.memset(dummy[:], 0.0)
            warmp = pwpool.tile([128, CHUNK], F32, tag="warm")
            for _ in range(10):
                nc.tensor.matmul(
                    warmp[:], dummy[:, 0:128], dummy[:], start=True, stop=True
                )

            def mm_chunk(pv, l0):
                for c in range(3):
                    nc.tensor.matmul(
                        pv,
                        wt[:, c * 128:(c + 1) * 128],
                        va[:, l0:l0 + CHUNK_ROWS, c:c + W],
                        start=(c == 0),
                        stop=False,
                    )
                # tap (ky2,kx2) is K=64 but issued as K=128 with zeroed
                # weight rows 64-127: a K=64 LDWEIGHTS cannot use the
                # background weight slot and serializes against the
                # in-flight matmul
                nc.tensor.matmul(
                    pv,
                    wt[:, 4 * 128:5 * 128],
                    va[:, l0 + 2:l0 + 2 + CHUNK_ROWS, 2:2 + W],
                    start=False,
                    stop=False,
                )
                nc.tensor.matmul(
                    pv,
                    wt[:, 3 * 128:4 * 128],
                    vb[:, l0 + 1:l0 + 1 + CHUNK_ROWS, 1:1 + W],
                    start=False,
                    stop=True,
                )

            # Merge: ACT evicts each PSUM granule to SBUF bf16, a sync-ring
            # DMA moves the low half across partitions into the output
            # buffer, DVE overlays the high half per mask. ALL bulk DMA
            # (loads, moves, stores) shares the sync HWDGE ring: the SDMA
            # engines serve rings with strict priority (gpsimd > sync >
            # scalar), so anything on a lower ring starves while loads
            # stream; one FIFO gives every transfer a bounded, timely slot.
            # Each tile's HBM store is deferred into the NEXT tile's merge:
            # by then its predicate pass has finished, so the store issues
            # without a semaphore wait that would stall the sequencer. The
            # last tile merges per 512-col chunk (stores on the by-then-idle
            # scalar ring) so the post-matmul tail is one short chain.
            prev_store = None
            for t in range(N_TILES):
                if t + 3 < N_TILES:
                    load_a(t + 3)
                if t + 2 < N_TILES:
                    load_b(t + 2)
                    copy_b(t + 2, *bblk[t + 2])
                if 1 <= t <= 3:
                    load_m(t)
                out_sb = opool.tile([COUT, TILE_POS], OUTDT, tag="osb")
                # the final two tiles merge per 512-col chunk so the
                # post-matmul drain proceeds in small interleaved units
                last = t >= N_TILES - 2
                ev = epool.tile([128, TILE_POS], OUTDT, tag="ev")
                for g in range(TILE_POS // GRAN):
                    pt = pspool.tile([128, GRAN], F32, tag="acc")
                    for cc in range(GRAN // CHUNK):
                        so = g * GRAN + cc * CHUNK
                        l0 = t * TROWS + so // W
                        pv = pt[:, cc * CHUNK:(cc + 1) * CHUNK].rearrange(
                            "p (r x) -> p r x", x=W
                        )
                        mm_chunk(pv, l0)
                        if last:
                            s = t * TILE_POS + so
                            nc.scalar.copy(
                                ev[:, so:so + CHUNK],
                                pt[:, cc * CHUNK:(cc + 1) * CHUNK],
                            )
                            nc.scalar.dma_start(
                                out_sb[:, so:so + CHUNK],
                                ev[64:128, so:so + CHUNK],
                            )
                            if so == 0 and prev_store is not None:
                                po, ps = prev_store
                                nc.sync.dma_start(
                                    out_d[:, ps:ps + TILE_POS], po[:]
                                )
                                prev_store = None
                            nc.vector.copy_predicated(
                                out_sb[:, so:so + CHUNK], mt[:, s:s + CHUNK],
                                ev[0:64, so:so + CHUNK],
                            )
                            if need_zero_fix:
                                nc.vector.copy_predicated(
                                    out_sb[:, so:so + CHUNK],
                                    mzt[:, s:s + CHUNK], zt[:, 0:CHUNK],
                                )
                            nc.sync.dma_start(
                                out_d[:, s:s + CHUNK], out_sb[:, so:so + CHUNK]
                            )
                    if not last:
                        so = g * GRAN
                        s = t * TILE_POS + so
                        nc.scalar.copy(ev[:, so:so + GRAN], pt[:])
                        # while loads stream, the sync FIFO is the only ring
                        # that isn't starved, so early moves ride it; once
                        # loads finish (~tile 5) the scalar ring issues moves
                        # engine-ordered right behind its own evict
                        mv = nc.scalar if t >= 5 else nc.sync
                        mv.dma_start(
                            out_sb[:, so:so + GRAN], ev[64:128, so:so + GRAN]
                        )
                        if g == 0 and prev_store is not None:
                            po, ps = prev_store
                            nc.sync.dma_start(out_d[:, ps:ps + TILE_POS], po[:])
                        nc.vector.copy_predicated(
                            out_sb[:, so:so + GRAN], mt[:, s:s + GRAN],
                            ev[0:64, so:so + GRAN],
                        )
                        if need_zero_fix:
                            nc.vector.copy_predicated(
                                out_sb[:, so:so + GRAN], mzt[:, s:s + GRAN],
                                zt[:, 0:GRAN],
                            )
                if not last:
                    prev_store = (out_sb, t * TILE_POS)

    nc.compile()
    return nc


def _prepare_host(inx, mask_idx, inv_mask_idx, high_w, low1_w, low2_w):
    inx = np.asarray(inx, dtype=np.float32)
    mask_idx = np.asarray(mask_idx).astype(np.int64)
    inv_mask_idx = np.asarray(inv_mask_idx).astype(np.int64)
    high_w = np.asarray(high_w, dtype=np.float32)
    low1_w = np.asarray(low1_w, dtype=np.float32)
    low2_w = np.asarray(low2_w, dtype=np.float32)

    # zero-padded flat images I [B, 64, 130*130] bf16
    inxp = np.zeros((B, CIN, HP, WP), np.float32)
    inxp[:, :, 1:-1, 1:-1] = inx
    iflat = inxp.reshape(B, CIN, L).astype(ml_dtypes.bfloat16)

    # hbma: partitions 0-63 = I, 64-127 = I shifted one row (+130)
    hbma = np.zeros((B, 128, L), ml_dtypes.bfloat16)
    hbma[:, 0:64] = iflat
    hbma[:, 64:128, 0:L - WP] = iflat[:, :, WP:]
    # hbmb: I shifted +129 (bufB top half; bottom half is copied on-chip)
    hbmb = np.zeros((B, 64, L), ml_dtypes.bfloat16)
    hbmb[:, :, 0:L - (WP - 1)] = iflat[:, :, WP - 1:]

    # fold the low branch: W_low[o, c, ky, kx] = sum_m w2[o, m] w1[m, c, ky, kx]
    w2 = low2_w.reshape(COUT, -1).astype(np.float64)
    wl = np.einsum("om,mckl->ockl", w2, low1_w.astype(np.float64)).astype(np.float32)
    wh = high_w

    # weight blob [128, 5*128] bf16; lhsT[k, m], m = output col (0-63 high,
    # 64-127 low-folded); k partition halves match the buffer layouts above
    blob = np.zeros((128, WBLK), ml_dtypes.bfloat16)
    for c in range(3):
        blk = blob[:, c * 128:(c + 1) * 128]
        blk[0:64, 0:64] = wh[:, :, 0, c].T
        blk[0:64, 64:128] = wl[:, :, 0, c].T
        blk[64:128, 0:64] = wh[:, :, 1, c].T
        blk[64:128, 64:128] = wl[:, :, 1, c].T
    b3 = blob[:, 3 * 128:4 * 128]
    b3[0:64, 0:64] = wh[:, :, 2, 0].T
    b3[0:64, 64:128] = wl[:, :, 2, 0].T
    b3[64:128, 0:64] = wh[:, :, 2, 1].T
    b3[64:128, 64:128] = wl[:, :, 2, 1].T
    b4 = blob[:, 4 * 128:5 * 128]
    b4[0:64, 0:64] = wh[:, :, 2, 2].T
    b4[0:64, 64:128] = wl[:, :, 2, 2].T

    ntotal = B * NPOS
    in_mask = np.zeros(ntotal, dtype=bool)
    in_inv = np.zeros(ntotal, dtype=bool)
    in_mask[mask_idx] = True
    in_inv[inv_mask_idx] = True
    # high wins only where inv doesn't claim (reference scatters inv last)
    m_high = in_mask & ~in_inv
    neither = ~(in_mask | in_inv)
    need_zero_fix = bool(neither.any())

    in_maps = []
    for b in range(B):
        sl = slice(b * NPOS, (b + 1) * NPOS)
        mh = np.ascontiguousarray(
            np.broadcast_to(
                m_high[sl].astype(np.uint8)[None, :], (COUT, NPOS)
            )
        )
        m = {"hbma": hbma[b], "hbmb": hbmb[b], "wblob": blob, "mhigh": mh}
        if need_zero_fix:
            m["mzero"] = np.ascontiguousarray(
                np.broadcast_to(
                    neither[sl].astype(np.uint8)[None, :], (COUT, NPOS)
                )
            )
        in_maps.append(m)
    return in_maps, need_zero_fix


def _run(inputs: dict, trace: bool = False):
    in_maps, need_zero_fix = _prepare_host(**inputs)
    nc = _build_program(need_zero_fix)
    res = run_bass_kernel_spmd(nc, in_maps, list(range(B)), trace=trace)
    out = np.stack(
        [np.asarray(res.results[b]["out"]).astype(np.float32).reshape(COUT, H, W)
         for b in range(B)]
    ).astype(np.float32)
    return out, res


def kernel(**inputs) -> np.ndarray:
    out, _ = _run(inputs, trace=False)
    return out
